# revision 28
# baseline (speedup 1.0000x reference)
"""Multi-head attention block on 8 Trainium2 NeuronCores, data-parallel over batch.

Per core (one batch element, S=1024 seq, E=1024 embed, H=16 heads, D=64),
all matmuls in bf16 (inputs cast host-side), fp32 PSUM accumulation:
  xT fed pre-transposed from the host (feature-major [E, S])
  qT/kT = W_pair.T @ xT  (feature-major) per head-pair, pipelined as PE filler
  V     = xT.T @ Wv      (seq-major) with a ones column appended -> V_aug
  scoresT[s2,s1] = kT.T @ qT  (two heads as K=64 row-tiles, overlapped on PE)
  expT = exp(0.125*scoresT)   (ACT eviction PSUM->SBUF, softmax w/o max-subtract;
                               logits are ~N(0,1.5) so exp cannot overflow fp32)
  PV: psum[66,512] = V_aug.T @ expT  -> rows 0..63 = outT unnorm, row 64 = rowsum
  normalize: outT = psum[0:64] * broadcast(reciprocal(psum[64]))
             (fast-approx reciprocal on DVE + GPSIMD partition broadcast)
  out = outT.T @ W_out + b_out

The scalar engine (ACT) is the attention-phase co-bottleneck (16 exp
activations per pair at ~1.15us each); it is kept exp-only — all copies and
broadcasts run on DVE/GPSIMD.  Weights are de-interleaved host-side:
reference W_qkv columns are (h, d, qkv) with qkv innermost; we feed wqk
(pair-blocked [q0q1k0k1...]) and wv ((h,d) order).
"""

import ml_dtypes
import numpy as np

import concourse.bacc as bacc
import concourse.bass as bass
import concourse.mybir as mybir
from concourse.bass_utils import run_bass_kernel_spmd
from concourse.tile import TileContext
from concourse.tile_rust import add_dep_helper

F32 = mybir.dt.float32
BF16 = mybir.dt.bfloat16
AF = mybir.ActivationFunctionType

S = 1024       # sequence length
E = 1024       # embed dim
H = 16         # heads
D = 64         # head dim
P = 128        # partitions
NP = 8         # head pairs
KT = E // P    # contraction tiles (8)
SM = S // P    # seq tiles of 128 (8)
NB = S // 512  # seq banks of 512 (2)
SCALE = 1.0 / np.sqrt(D)


def build_nc():
    nc = bacc.Bacc(trn_type="TRN2", target_bir_lowering=False)
    xt = nc.dram_tensor("xt", [E, S], BF16, kind="ExternalInput")
    wqk = nc.dram_tensor("wqk", [E, 2 * E], BF16, kind="ExternalInput")
    wv = nc.dram_tensor("wv", [E, E], BF16, kind="ExternalInput")
    bqk = nc.dram_tensor("bqk", [2 * E], F32, kind="ExternalInput")
    bv = nc.dram_tensor("bv", [E], F32, kind="ExternalInput")
    wout = nc.dram_tensor("wout", [E, E], BF16, kind="ExternalInput")
    bout = nc.dram_tensor("bout", [E], F32, kind="ExternalInput")
    out = nc.dram_tensor("out", [S, E], F32, kind="ExternalOutput")

    with TileContext(nc) as tc:
        with (
            tc.tile_pool(name="const", bufs=1) as constp,
            tc.tile_pool(name="persist", bufs=1) as pers,
            tc.tile_pool(name="psum", bufs=1, space="PSUM") as psp,
        ):
            # ---- constants ----
            ones = constp.tile([1, 512], F32, tag="ones")
            nc.vector.memset(ones[:], 1.0)
            onespp = constp.tile([P, 2 * H], F32, tag="onespp")
            nc.vector.memset(onespp[:], 1.0)
            warm = constp.tile([P, 512], BF16, tag="warm")
            nc.vector.memset(warm[:], 0.0)
            # per-partition bias columns for q/k (column p*2+j is the bias for
            # pair p's q (j=0) / k (j=1) feature block)
            bcols = constp.tile([P, 2 * NP], F32, tag="bcols")

            # ---- persistent arrays ----
            xTall = pers.tile([P, KT, S], BF16, tag="xtall", name="xTall")
            xT = [xTall[:, k] for k in range(KT)]
            vaug = [pers.tile([P, H, D + 2], BF16, tag=f"va{m}", name=f"vaug{m}")
                    for m in range(SM)]
            outT = [pers.tile([P, S], BF16, tag=f"ot{p}", name=f"outT{p}")
                    for p in range(NP)]

            # broadcast biases for free-dim adds (V and final projections)
            bvb = constp.tile([P, E], F32, tag="bvb")
            boutb = constp.tile([P, E], F32, tag="boutb")
            with (
                tc.tile_pool(name="ph0", bufs=1) as ph0,
                tc.tile_pool(name="ph2", bufs=1) as ph2,
                tc.tile_pool(name="ph3", bufs=1) as ph3,
            ):
                # ---- input DMAs, spread across sync/scalar queues ----
                # Per-DMA issue costs ~0.7us on a queue, and V-proj m-tile m
                # needs xT chunk m//2 of every k plus wv bank 0 — interleave
                # so V can start ~14us and never starves afterwards.
                wvk = [[ph0.tile([P, 512], BF16, tag=f"wv{n}_{k}", name="wvk")
                        for k in range(KT)] for n in range(2)]

                def dma_xt_chunk(c, parity, eng):
                    ch = slice(c * 256, (c + 1) * 256)
                    for k in range(parity, KT, 2):
                        eng.dma_start(xT[k][:, ch], xt.ap()[bass.ts(k, P), ch])

                def dma_wv(n, ks, eng):
                    for k in ks:
                        eng.dma_start(
                            wvk[n][k][:], wv.ap()[bass.ts(k, P), bass.ts(n, 512)])

                dma_xt_chunk(0, 0, nc.sync)
                dma_xt_chunk(0, 1, nc.scalar)
                dma_wv(0, range(0, 4), nc.sync)
                dma_wv(0, range(4, 8), nc.scalar)
                for c in range(1, 4):
                    dma_xt_chunk(c, 0, nc.sync)
                    dma_xt_chunk(c, 1, nc.scalar)
                dma_wv(1, range(0, 4), nc.sync)
                dma_wv(1, range(4, 8), nc.scalar)
                # small bias DMAs on the gpsimd (SWDGE) queue
                bvr = ph0.tile([1, E], F32, tag="bvr")
                nc.gpsimd.dma_start(bvr[:], bv.ap()[None, :])
                botr = ph0.tile([1, E], F32, tag="botr")
                nc.gpsimd.dma_start(botr[:], bout.ap()[None, :])
                nc.gpsimd.dma_start(
                    bcols[:], bqk.ap().rearrange("(f p) -> p f", p=P))

                def load_wq(p):
                    wq = []
                    for k in range(KT):
                        w = ph2.tile([P, 256], BF16, tag="wqk", bufs=16, name="wqk")
                        nc.sync.dma_start(
                            w[:], wqk.ap()[bass.ts(k, P), bass.ts(p, 256)])
                        wq.append(w)
                    return wq

                wq0 = load_wq(0)

                # ---- PE warmup: junk matmuls so HAM un-throttles before V
                # and the PE never idles a full MID window while DMAs land ----
                for g in range(3):
                    wp = psp.tile([P, 512], F32, tag="mm", bufs=2, name="warmps")
                    for i in range(8):
                        nc.tensor.matmul(wp[:], warm[:, 0:P], warm[:],
                                         start=(i == 0), stop=(i == 7))

                # bias row -> all-partition broadcasts (GPSIMD, off the PE)
                nc.gpsimd.partition_broadcast(bvb[:], bvr[:])
                nc.gpsimd.partition_broadcast(boutb[:], botr[:])

                # ---- phase 1: V = x @ Wv (+bv), into vaug with ones column ----
                for m in range(SM):
                    nc.vector.tensor_copy(
                        vaug[m][:, :, D:D + 2],
                        onespp[:].rearrange("p (h t) -> p h t", h=H))
                for n in range(2):
                    for m in range(SM):
                        pv = psp.tile([P, 512], F32, tag="mm", bufs=2, name="pvps")
                        for k in range(KT):
                            nc.tensor.matmul(
                                pv[:], xT[k][:, bass.ts(m, P)], wvk[n][k][:],
                                start=(k == 0), stop=(k == KT - 1))
                        nc.vector.tensor_add(
                            vaug[m][:, bass.ts(n, 8), 0:D],
                            pv[:].rearrange("p (h d) -> p h d", h=8),
                            bvb[:, bass.ts(n, 512)].rearrange("p (h d) -> p h d", h=8))

                # ---- phase 2: attention, software-pipelined over head pairs ----
                # Iteration p computes attention for pair p while projecting
                # qt/kt for pair p+1 (proj matmuls interleaved into the scores
                # loop so PE has independent work while ACT evicts exp tiles).
                def load_wot(n):
                    cs = bass.ts(n, 512)
                    wot = []
                    for k in range(KT):
                        w = ph3.tile([P, 512], BF16, tag=f"wo{k}", bufs=2,
                                     name="wot")
                        nc.scalar.dma_start(w[:], wout.ap()[bass.ts(k, P), cs])
                        wot.append(w)
                    return wot

                def alloc_qkt():
                    qt = ph2.tile([P, S], BF16, tag="qt", bufs=2, name="qt")
                    kt = ph2.tile([P, S], BF16, tag="kt", bufs=2, name="kt")
                    return qt, kt

                def proj_mms(p, wq, qt, kt):
                    """Generator yielding proj matmuls + evictions.

                    k-outer / bank-inner: consecutive matmuls share the
                    stationary wq tile, halving the weight loads."""
                    for which in range(2):  # 0 = q, 1 = k
                        ws = slice(which * P, (which + 1) * P)
                        dst = qt if which == 0 else kt
                        bc = bcols[:, 2 * p + which:2 * p + which + 1]
                        ps = [psp.tile([P, 512], F32, tag="mm", bufs=2,
                                       name="pproj") for _ in range(NB)]
                        for k in range(KT):
                            for n in range(NB):
                                nc.tensor.matmul(
                                    ps[n][:], wq[k][:, ws],
                                    xT[k][:, bass.ts(n, 512)],
                                    start=(k == 0), stop=(k == KT - 1))
                                yield
                        for n in range(NB):
                            nc.vector.tensor_scalar_add(
                                dst[:, bass.ts(n, 512)], ps[n][:], bc)
                    while True:
                        yield

                def drain(gen, n):
                    for _ in range(n):
                        next(gen)

                # Final projection tiles as open/close chains: k=0..6 needs
                # only pairs 0-6 outT (ready before pair 7), so those matmuls
                # fill pair 7's attention windows; the k=7 close + bias-add +
                # store happen once pair 7's outT bank is normalized.
                _open_pf = {}

                def final_m_open(m):
                    # k=0..6 for both feature banks of out rows m*128..;
                    # consecutive matmuls share the stationary outT tile.
                    # Needs only pairs 0-6 outT, so this runs before pair 7's
                    # normalize of the m//4 bank.
                    pf = [psp.tile([P, 512], F32, tag="mm", bufs=2, name="pf")
                          for _ in range(2)]
                    _open_pf[m] = pf
                    for k in range(KT - 1):
                        for n in range(2):
                            wot = wot0 if n == 0 else wot1
                            nc.tensor.matmul(
                                pf[n][:], outT[k][:, bass.ts(m, P)], wot[k][:],
                                start=(k == 0), stop=False)
                            yield

                def final_m_close(m):
                    pf = _open_pf.pop(m)
                    for n in range(2):
                        wot = wot0 if n == 0 else wot1
                        nc.tensor.matmul(
                            pf[n][:], outT[KT - 1][:, bass.ts(m, P)],
                            wot[KT - 1][:], start=False, stop=True)
                        yield
                    for n in range(2):
                        cs = bass.ts(n, 512)
                        osb = ph3.tile([P, 512], F32, tag="osb", bufs=3,
                                       name="osb")
                        nc.vector.tensor_add(osb[:], pf[n][:], boutb[:, cs])
                        nc.sync.dma_start(out.ap()[bass.ts(m, P), cs], osb[:])

                def final_b0_prefix():
                    # pair-7 bank-0 filler: open k0-6 chains for seq-tile 0
                    yield from final_m_open(0)
                    while True:
                        yield

                def final_rest():
                    # after pair-7 bank-0 normalize: close the open chains,
                    # then stream the rest of the first-half tiles
                    yield from final_m_close(0)
                    for m in range(1, SM // 2):
                        yield from final_m_open(m)
                        yield from final_m_close(m)
                    while True:
                        yield

                wq = wq0
                qt, kt = alloc_qkt()
                drain(proj_mms(0, wq, qt, kt), 40)

                for p in range(NP):
                    if p + 1 < NP:
                        wq_n = load_wq(p + 1)
                        qt_n, kt_n = alloc_qkt()
                        filler = proj_mms(p + 1, wq_n, qt_n, kt_n)
                    else:
                        filler = final_b0_prefix()

                    # Software pipeline over 8 groups (2 banks x 4 m-iters):
                    # group g emits scores+exp for (bank g//4, m=2(g%4)); the
                    # PV matmuls for group g-1 follow right after, so the PE
                    # queue head is always ready when PE reaches it and ACT's
                    # exp chain never starves between banks.
                    expAn = [None] * NB
                    expBn = [None] * NB
                    poAn = [None] * NB
                    poBn = [None] * NB

                    def emit_pv(n, m):
                        for j in range(2):
                            nc.tensor.matmul(
                                poAn[n][:], vaug[m + j][:, 2 * p, :],
                                expAn[n][:, m + j],
                                start=(m + j == 0), stop=(m + j == SM - 1))
                            nc.tensor.matmul(
                                poBn[n][:], vaug[m + j][:, 2 * p + 1, :],
                                expBn[n][:, m + j],
                                start=(m + j == 0), stop=(m + j == SM - 1))

                    def emit_norm(n):
                        cs = bass.ts(n, 512)
                        for h, po in ((0, poAn[n]), (1, poBn[n])):
                            # evict [66,512] to SBUF fast so the PSUM slot
                            # frees; normalize out of SBUF (DVE, not ACT —
                            # the ACT queue must stay exp-only)
                            pvt = ph2.tile([D, 512], F32, tag="pvt",
                                           bufs=4, name="pvt")
                            nc.vector.tensor_copy(pvt[:], po[0:D, :])
                            rs = ph2.tile([1, 512], F32, tag="rs", bufs=4,
                                          name="rs")
                            nc.vector.tensor_copy(rs[:], po[D:D + 1, :])
                            drain(filler, 4)
                            rec = ph2.tile([1, 512], F32, tag="rec", bufs=4,
                                           name="rec")
                            nc.vector.reciprocal_approx_fast(rec[:], rs[:])
                            rb = ph2.tile([D, 512], F32, tag="rb", bufs=4,
                                          name="rb")
                            nc.gpsimd.partition_broadcast(rb[:], rec[:])
                            nc.vector.tensor_mul(
                                outT[p][h * D:(h + 1) * D, cs],
                                pvt[:], rb[:])

                    for g in range(9):
                        if g < 8:
                            n, it = g // 4, g % 4
                            if it == 0:
                                cs = bass.ts(n, 512)
                                expAn[n] = ph2.tile([P, SM, 512], BF16,
                                                    tag="expA", bufs=2,
                                                    name="expA")
                                expBn[n] = ph2.tile([P, SM, 512], BF16,
                                                    tag="expB", bufs=2,
                                                    name="expB")
                                poAn[n] = psp.tile([D + 2, 512], F32,
                                                   tag="pv", bufs=2, name="poA")
                                poBn[n] = psp.tile([D + 2, 512], F32,
                                                   tag="pv", bufs=2, name="poB")
                            m = 2 * it
                            psA = psp.tile([P, 2, 512], F32, tag="sc", bufs=2,
                                           name="psA")
                            psB = psp.tile([P, 2, 512], F32, tag="sc", bufs=2,
                                           name="psB")
                            prev = None
                            for j in range(2):
                                ms = bass.ts(m + j, P)
                                ia = nc.tensor.matmul(
                                    psA[:, j], kt[0:D, ms], qt[0:D, cs])
                                ib = nc.tensor.matmul(
                                    psB[:, j], kt[D:P, ms], qt[D:P, cs])
                                # chain so the two half-array (row-tiled)
                                # matmuls issue back-to-back and overlap
                                if prev is not None:
                                    add_dep_helper(ia.ins, prev.ins, sync=False,
                                                   reason="pair scores order")
                                add_dep_helper(ib.ins, ia.ins, sync=False,
                                               reason="pair scores order")
                                prev = ib
                            nc.scalar.activation(
                                expAn[n][:, m:m + 2], psA[:], AF.Exp,
                                scale=SCALE)
                            nc.scalar.activation(
                                expBn[n][:, m:m + 2], psB[:], AF.Exp,
                                scale=SCALE)
                            drain(filler, 4 if g < 6 else 2)
                        if g >= 1:
                            pn, pit = (g - 1) // 4, (g - 1) % 4
                            emit_pv(pn, 2 * pit)
                            if pit == 3:
                                emit_norm(pn)
                                if p == NP - 1 and pn == 0:
                                    # outT bank 0 is complete for all pairs —
                                    # the first-half projection tiles become
                                    # the PE filler for pair 7's bank 1
                                    filler = final_rest()
                    if p + 1 < NP:
                        drain(filler, 64)
                        wq, qt, kt = wq_n, qt_n, kt_n
                    if p == NP - 2:
                        wot0 = load_wot(0)
                        wot1 = load_wot(1)

                # ---- phase 3 (second half; first half emitted during pair 7)
                drain(filler, 200)
                for m in range(SM // 2, SM):
                    for _ in final_m_open(m):
                        pass
                    for _ in final_m_close(m):
                        pass

    nc.finalize()
    return nc


_NC = None


def _get_nc():
    global _NC
    if _NC is None:
        _NC = build_nc()
    return _NC


def _prep_weights(W_qkv, b_qkv):
    # reference column order is (h, d, qkv) with qkv innermost
    W = np.asarray(W_qkv, dtype=np.float32).reshape(E, H, D, 3)
    b = np.asarray(b_qkv, dtype=np.float32).reshape(H, D, 3)
    Wq = W[..., 0].reshape(E, E)
    Wk = W[..., 1].reshape(E, E)
    Wv = W[..., 2].reshape(E, E)
    bq = b[..., 0].reshape(E)
    bk = b[..., 1].reshape(E)
    bv = b[..., 2].reshape(E)
    wqk = np.empty((E, 2 * E), dtype=np.float32)
    bqk = np.empty(2 * E, dtype=np.float32)
    for p in range(NP):
        wqk[:, p * 256:p * 256 + P] = Wq[:, p * P:(p + 1) * P]
        wqk[:, p * 256 + P:(p + 1) * 256] = Wk[:, p * P:(p + 1) * P]
        bqk[p * 256:p * 256 + P] = bq[p * P:(p + 1) * P]
        bqk[p * 256 + P:(p + 1) * 256] = bk[p * P:(p + 1) * P]
    return wqk, np.ascontiguousarray(Wv), bqk, bv


def kernel(x, W_qkv, b_qkv, W_out, b_out, _trace=False, _tmpdir=None):
    bf = ml_dtypes.bfloat16
    x = np.asarray(x, dtype=np.float32).astype(bf)
    wqk, wv, bqk, bv = _prep_weights(W_qkv, b_qkv)
    wqk = wqk.astype(bf)
    wv = wv.astype(bf)
    wout = np.ascontiguousarray(
        np.asarray(W_out, dtype=np.float32).astype(bf))
    bout = np.ascontiguousarray(np.asarray(b_out, dtype=np.float32))
    nc = _get_nc()
    in_maps = [
        {"xt": np.ascontiguousarray(x[i].T), "wqk": wqk, "wv": wv,
         "bqk": bqk, "bv": bv, "wout": wout, "bout": bout}
        for i in range(x.shape[0])
    ]
    res = run_bass_kernel_spmd(
        nc, in_maps, core_ids=list(range(x.shape[0])),
        trace=_trace, tmpdir=_tmpdir)
    outp = np.stack([rr["out"] for rr in res.results], axis=0)
    kernel.last_result = res
    return outp


# revision 30
# speedup vs baseline: 1.0464x; 1.0464x over previous
"""Multi-head attention block on 8 Trainium2 NeuronCores, data-parallel over batch.

Per core (one batch element, S=1024 seq, E=1024 embed, H=16 heads, D=64),
all matmuls in bf16 (inputs cast host-side), fp32 PSUM accumulation:
  xT fed pre-transposed from the host (feature-major [E, S])
  qT/kT = W_pair.T @ xT  (feature-major) per head-pair, pipelined as PE filler
  V     = xT.T @ Wv      (seq-major) with a ones column appended -> V_aug
  scoresT[s2,s1] = kT.T @ qT  (two heads as K=64 row-tiles, overlapped on PE)
  expT = exp(0.125*scoresT)   (ACT eviction PSUM->SBUF, softmax w/o max-subtract;
                               logits are ~N(0,1.5) so exp cannot overflow fp32)
  PV: psum[66,512] = V_aug.T @ expT  -> rows 0..63 = outT unnorm, row 64 = rowsum
  normalize: outT = psum[0:64] * broadcast(reciprocal(psum[64]))
             (fast-approx reciprocal on DVE + GPSIMD partition broadcast)
  out = outT.T @ W_out + b_out

The scalar engine (ACT) is the attention-phase co-bottleneck (16 exp
activations per pair at ~1.15us each); it is kept exp-only — all copies and
broadcasts run on DVE/GPSIMD.  Weights are de-interleaved host-side:
reference W_qkv columns are (h, d, qkv) with qkv innermost; we feed wqk
(pair-blocked [q0q1k0k1...]) and wv ((h,d) order).
"""

import ml_dtypes
import numpy as np

import concourse.bacc as bacc
import concourse.bass as bass
import concourse.mybir as mybir
from concourse.bass_utils import run_bass_kernel_spmd
from concourse.tile import TileContext
from concourse.tile_rust import add_dep_helper

F32 = mybir.dt.float32
BF16 = mybir.dt.bfloat16
AF = mybir.ActivationFunctionType

S = 1024       # sequence length
E = 1024       # embed dim
H = 16         # heads
D = 64         # head dim
P = 128        # partitions
NP = 8         # head pairs
KT = E // P    # contraction tiles (8)
SM = S // P    # seq tiles of 128 (8)
NB = S // 512  # seq banks of 512 (2)
SCALE = 1.0 / np.sqrt(D)


def build_nc():
    nc = bacc.Bacc(trn_type="TRN2", target_bir_lowering=False)
    xt = nc.dram_tensor("xt", [E, S], BF16, kind="ExternalInput")
    wqk = nc.dram_tensor("wqk", [E, 2 * E], BF16, kind="ExternalInput")
    wv = nc.dram_tensor("wv", [E, E], BF16, kind="ExternalInput")
    bqk = nc.dram_tensor("bqk", [2 * E], F32, kind="ExternalInput")
    bv = nc.dram_tensor("bv", [E], F32, kind="ExternalInput")
    wout = nc.dram_tensor("wout", [E, E], BF16, kind="ExternalInput")
    bout = nc.dram_tensor("bout", [E], F32, kind="ExternalInput")
    out = nc.dram_tensor("out", [S, E], F32, kind="ExternalOutput")

    with TileContext(nc) as tc:
        with (
            tc.tile_pool(name="const", bufs=1) as constp,
            tc.tile_pool(name="persist", bufs=1) as pers,
            tc.tile_pool(name="psum", bufs=1, space="PSUM") as psp,
        ):
            # ---- constants ----
            ones = constp.tile([1, 512], F32, tag="ones")
            nc.vector.memset(ones[:], 1.0)
            onespp = constp.tile([P, 2 * H], F32, tag="onespp")
            nc.vector.memset(onespp[:], 1.0)
            warm = constp.tile([P, 512], BF16, tag="warm")
            nc.vector.memset(warm[:], 0.0)
            # per-partition bias columns for q/k (column p*2+j is the bias for
            # pair p's q (j=0) / k (j=1) feature block)
            bcols = constp.tile([P, 2 * NP], F32, tag="bcols")

            # ---- persistent arrays ----
            xTall = pers.tile([P, KT, S], BF16, tag="xtall", name="xTall")
            xT = [xTall[:, k] for k in range(KT)]
            vaug = [pers.tile([P, H, D + 2], BF16, tag=f"va{m}", name=f"vaug{m}")
                    for m in range(SM)]
            outT = [pers.tile([P, S], BF16, tag=f"ot{p}", name=f"outT{p}")
                    for p in range(NP)]

            # broadcast biases for free-dim adds (V and final projections)
            bvb = constp.tile([P, E], F32, tag="bvb")
            boutb = constp.tile([P, E], F32, tag="boutb")
            with (
                tc.tile_pool(name="ph0", bufs=1) as ph0,
                tc.tile_pool(name="ph2", bufs=1) as ph2,
                tc.tile_pool(name="ph3", bufs=1) as ph3,
            ):
                # ---- input DMAs, spread across sync/scalar queues ----
                # Per-DMA issue costs ~0.7us on a queue, and V-proj m-tile m
                # needs xT chunk m//2 of every k plus wv bank 0 — interleave
                # so V can start ~14us and never starves afterwards.
                wvk = [[ph0.tile([P, 512], BF16, tag=f"wv{n}_{k}", name="wvk")
                        for k in range(KT)] for n in range(2)]

                def dma_xt_chunk(c, parity, eng):
                    ch = slice(c * 256, (c + 1) * 256)
                    for k in range(parity, KT, 2):
                        eng.dma_start(xT[k][:, ch], xt.ap()[bass.ts(k, P), ch])

                def dma_wv(n, ks, eng):
                    for k in ks:
                        eng.dma_start(
                            wvk[n][k][:], wv.ap()[bass.ts(k, P), bass.ts(n, 512)])

                dma_xt_chunk(0, 0, nc.sync)
                dma_xt_chunk(0, 1, nc.scalar)
                dma_wv(0, range(0, 4), nc.sync)
                dma_wv(0, range(4, 8), nc.scalar)
                dma_xt_chunk(1, 0, nc.sync)
                dma_xt_chunk(1, 1, nc.scalar)
                dma_wv(1, range(0, 4), nc.sync)
                dma_wv(1, range(4, 8), nc.scalar)
                dma_xt_chunk(2, 0, nc.sync)
                dma_xt_chunk(2, 1, nc.scalar)
                dma_xt_chunk(3, 0, nc.sync)
                dma_xt_chunk(3, 1, nc.scalar)
                # small bias DMAs on the gpsimd (SWDGE) queue
                bvr = ph0.tile([1, E], F32, tag="bvr")
                nc.gpsimd.dma_start(bvr[:], bv.ap()[None, :])
                botr = ph0.tile([1, E], F32, tag="botr")
                nc.gpsimd.dma_start(botr[:], bout.ap()[None, :])
                nc.gpsimd.dma_start(
                    bcols[:], bqk.ap().rearrange("(f p) -> p f", p=P))

                def load_wq(p):
                    wq = []
                    for k in range(KT):
                        w = ph2.tile([P, 256], BF16, tag="wqk", bufs=16, name="wqk")
                        nc.sync.dma_start(
                            w[:], wqk.ap()[bass.ts(k, P), bass.ts(p, 256)])
                        wq.append(w)
                    return wq

                wq0 = load_wq(0)

                # ---- PE warmup: junk matmuls so HAM un-throttles before V
                # and the PE never idles a full MID window while DMAs land ----
                for g in range(3):
                    wp = psp.tile([P, 512], F32, tag="mm", bufs=2, name="warmps")
                    for i in range(8):
                        nc.tensor.matmul(wp[:], warm[:, 0:P], warm[:],
                                         start=(i == 0), stop=(i == 7))

                # bias row -> all-partition broadcasts (GPSIMD, off the PE)
                nc.gpsimd.partition_broadcast(bvb[:], bvr[:])
                nc.gpsimd.partition_broadcast(boutb[:], botr[:])

                # ---- phase 1: V = x @ Wv (+bv), into vaug with ones column ----
                for m in range(SM):
                    nc.vector.tensor_copy(
                        vaug[m][:, :, D:D + 2],
                        onespp[:].rearrange("p (h t) -> p h t", h=H))
                for n in range(2):
                    for m in range(SM):
                        pv = psp.tile([P, 512], F32, tag="mm", bufs=2, name="pvps")
                        for k in range(KT):
                            nc.tensor.matmul(
                                pv[:], xT[k][:, bass.ts(m, P)], wvk[n][k][:],
                                start=(k == 0), stop=(k == KT - 1))
                        nc.vector.tensor_add(
                            vaug[m][:, bass.ts(n, 8), 0:D],
                            pv[:].rearrange("p (h d) -> p h d", h=8),
                            bvb[:, bass.ts(n, 512)].rearrange("p (h d) -> p h d", h=8))

                # ---- phase 2: attention, software-pipelined over head pairs ----
                # Iteration p computes attention for pair p while projecting
                # qt/kt for pair p+1 (proj matmuls interleaved into the scores
                # loop so PE has independent work while ACT evicts exp tiles).
                def load_wot(n):
                    cs = bass.ts(n, 512)
                    wot = []
                    for k in range(KT):
                        w = ph3.tile([P, 512], BF16, tag=f"wo{k}", bufs=2,
                                     name="wot")
                        nc.scalar.dma_start(w[:], wout.ap()[bass.ts(k, P), cs])
                        wot.append(w)
                    return wot

                def alloc_qkt():
                    qt = ph2.tile([P, S], BF16, tag="qt", bufs=2, name="qt")
                    kt = ph2.tile([P, S], BF16, tag="kt", bufs=2, name="kt")
                    return qt, kt

                def proj_mms(p, wq, qt, kt):
                    """Generator yielding groups of proj matmuls + evictions."""
                    for which in range(2):  # 0 = q, 1 = k
                        ws = slice(which * P, (which + 1) * P)
                        dst = qt if which == 0 else kt
                        bc = bcols[:, 2 * p + which:2 * p + which + 1]
                        for n in range(NB):
                            cs = bass.ts(n, 512)
                            ps = psp.tile([P, 512], F32, tag="mm", bufs=2,
                                          name="pproj")
                            for k in range(KT):
                                nc.tensor.matmul(
                                    ps[:], wq[k][:, ws], xT[k][:, cs],
                                    start=(k == 0), stop=(k == KT - 1))
                                yield
                            nc.vector.tensor_scalar_add(dst[:, cs], ps[:], bc)
                    while True:
                        yield

                def drain(gen, n):
                    for _ in range(n):
                        next(gen)

                # Final projection tiles as open/close chains: k=0..6 needs
                # only pairs 0-6 outT (ready before pair 7), so those matmuls
                # fill pair 7's attention windows; the k=7 close + bias-add +
                # store happen once pair 7's outT bank is normalized.
                _open_pf = {}

                def final_tile_open(m, n):
                    pf = psp.tile([P, 512], F32, tag="mm", bufs=2, name="pf")
                    _open_pf[(m, n)] = pf
                    wot = wot0 if n == 0 else wot1
                    for k in range(KT - 1):
                        nc.tensor.matmul(
                            pf[:], outT[k][:, bass.ts(m, P)], wot[k][:],
                            start=(k == 0), stop=False)
                        yield

                def final_tile_close(m, n):
                    pf = _open_pf.pop((m, n))
                    wot = wot0 if n == 0 else wot1
                    cs = bass.ts(n, 512)
                    nc.tensor.matmul(
                        pf[:], outT[KT - 1][:, bass.ts(m, P)], wot[KT - 1][:],
                        start=False, stop=True)
                    yield
                    osb = ph3.tile([P, 512], F32, tag="osb", bufs=3,
                                   name="osb")
                    nc.vector.tensor_add(osb[:], pf[:], boutb[:, cs])
                    nc.sync.dma_start(out.ap()[bass.ts(m, P), cs], osb[:])

                def final_b0_prefix():
                    # pair-7 bank-0 filler: two open k0-6 chains (14 matmuls)
                    yield from final_tile_open(0, 0)
                    yield from final_tile_open(1, 0)
                    while True:
                        yield

                def final_rest():
                    # after pair-7 bank-0 normalize: close the open chains,
                    # then stream the rest of the first-half tiles
                    yield from final_tile_close(0, 0)
                    yield from final_tile_close(1, 0)
                    for m, n in [(2, 0), (3, 0), (0, 1), (1, 1),
                                 (2, 1), (3, 1)]:
                        yield from final_tile_open(m, n)
                        yield from final_tile_close(m, n)
                    while True:
                        yield

                wq = wq0
                qt, kt = alloc_qkt()
                drain(proj_mms(0, wq, qt, kt), 40)

                for p in range(NP):
                    if p + 1 < NP:
                        wq_n = load_wq(p + 1)
                        qt_n, kt_n = alloc_qkt()
                        filler = proj_mms(p + 1, wq_n, qt_n, kt_n)
                    else:
                        filler = final_b0_prefix()

                    # Software pipeline over 8 groups (2 banks x 4 m-iters):
                    # group g emits scores+exp for (bank g//4, m=2(g%4)); the
                    # PV matmuls for group g-1 follow right after, so the PE
                    # queue head is always ready when PE reaches it and ACT's
                    # exp chain never starves between banks.
                    expAn = [None] * NB
                    expBn = [None] * NB
                    poAn = [None] * NB
                    poBn = [None] * NB

                    def emit_pv(n, m):
                        for j in range(2):
                            nc.tensor.matmul(
                                poAn[n][:], vaug[m + j][:, 2 * p, :],
                                expAn[n][:, m + j],
                                start=(m + j == 0), stop=(m + j == SM - 1))
                            nc.tensor.matmul(
                                poBn[n][:], vaug[m + j][:, 2 * p + 1, :],
                                expBn[n][:, m + j],
                                start=(m + j == 0), stop=(m + j == SM - 1))

                    def emit_norm(n):
                        cs = bass.ts(n, 512)
                        for h, po in ((0, poAn[n]), (1, poBn[n])):
                            # evict [66,512] to SBUF fast so the PSUM slot
                            # frees; normalize out of SBUF (DVE, not ACT —
                            # the ACT queue must stay exp-only)
                            pvt = ph2.tile([D, 512], F32, tag="pvt",
                                           bufs=4, name="pvt")
                            nc.vector.tensor_copy(pvt[:], po[0:D, :])
                            rs = ph2.tile([1, 512], F32, tag="rs", bufs=4,
                                          name="rs")
                            nc.vector.tensor_copy(rs[:], po[D:D + 1, :])
                            drain(filler, 4)
                            rec = ph2.tile([1, 512], F32, tag="rec", bufs=4,
                                           name="rec")
                            nc.vector.reciprocal_approx_fast(rec[:], rs[:])
                            rb = ph2.tile([D, 512], F32, tag="rb", bufs=4,
                                          name="rb")
                            nc.gpsimd.partition_broadcast(rb[:], rec[:])
                            nc.vector.tensor_mul(
                                outT[p][h * D:(h + 1) * D, cs],
                                pvt[:], rb[:])

                    for g in range(9):
                        if g < 8:
                            n, it = g // 4, g % 4
                            if it == 0:
                                cs = bass.ts(n, 512)
                                expAn[n] = ph2.tile([P, SM, 512], BF16,
                                                    tag="expA", bufs=2,
                                                    name="expA")
                                expBn[n] = ph2.tile([P, SM, 512], BF16,
                                                    tag="expB", bufs=2,
                                                    name="expB")
                                poAn[n] = psp.tile([D + 2, 512], F32,
                                                   tag="pv", bufs=2, name="poA")
                                poBn[n] = psp.tile([D + 2, 512], F32,
                                                   tag="pv", bufs=2, name="poB")
                            m = 2 * it
                            psA = psp.tile([P, 2, 512], F32, tag="sc", bufs=2,
                                           name="psA")
                            psB = psp.tile([P, 2, 512], F32, tag="sc", bufs=2,
                                           name="psB")
                            prev = None
                            for j in range(2):
                                ms = bass.ts(m + j, P)
                                ia = nc.tensor.matmul(
                                    psA[:, j], kt[0:D, ms], qt[0:D, cs])
                                ib = nc.tensor.matmul(
                                    psB[:, j], kt[D:P, ms], qt[D:P, cs])
                                # chain so the two half-array (row-tiled)
                                # matmuls issue back-to-back and overlap
                                if prev is not None:
                                    add_dep_helper(ia.ins, prev.ins, sync=False,
                                                   reason="pair scores order")
                                add_dep_helper(ib.ins, ia.ins, sync=False,
                                               reason="pair scores order")
                                prev = ib
                            nc.scalar.activation(
                                expAn[n][:, m:m + 2], psA[:], AF.Exp,
                                scale=SCALE)
                            nc.scalar.activation(
                                expBn[n][:, m:m + 2], psB[:], AF.Exp,
                                scale=SCALE)
                            drain(filler, 4 if g < 6 else 2)
                        if g >= 1:
                            pn, pit = (g - 1) // 4, (g - 1) % 4
                            emit_pv(pn, 2 * pit)
                            if pit == 3:
                                emit_norm(pn)
                                if p == NP - 1 and pn == 0:
                                    # outT bank 0 is complete for all pairs —
                                    # the first-half projection tiles become
                                    # the PE filler for pair 7's bank 1
                                    filler = final_rest()
                    if p + 1 < NP:
                        drain(filler, 64)
                        wq, qt, kt = wq_n, qt_n, kt_n
                    if p == NP - 2:
                        wot0 = load_wot(0)
                        wot1 = load_wot(1)

                # ---- phase 3 (second half; first half emitted during pair 7)
                drain(filler, 200)
                for m in range(SM // 2, SM):
                    for n in range(2):
                        for _ in final_tile_open(m, n):
                            pass
                        for _ in final_tile_close(m, n):
                            pass

    nc.finalize()
    return nc


_NC = None


def _get_nc():
    global _NC
    if _NC is None:
        _NC = build_nc()
    return _NC


def _prep_weights(W_qkv, b_qkv):
    # reference column order is (h, d, qkv) with qkv innermost
    W = np.asarray(W_qkv, dtype=np.float32).reshape(E, H, D, 3)
    b = np.asarray(b_qkv, dtype=np.float32).reshape(H, D, 3)
    Wq = W[..., 0].reshape(E, E)
    Wk = W[..., 1].reshape(E, E)
    Wv = W[..., 2].reshape(E, E)
    bq = b[..., 0].reshape(E)
    bk = b[..., 1].reshape(E)
    bv = b[..., 2].reshape(E)
    wqk = np.empty((E, 2 * E), dtype=np.float32)
    bqk = np.empty(2 * E, dtype=np.float32)
    for p in range(NP):
        wqk[:, p * 256:p * 256 + P] = Wq[:, p * P:(p + 1) * P]
        wqk[:, p * 256 + P:(p + 1) * 256] = Wk[:, p * P:(p + 1) * P]
        bqk[p * 256:p * 256 + P] = bq[p * P:(p + 1) * P]
        bqk[p * 256 + P:(p + 1) * 256] = bk[p * P:(p + 1) * P]
    return wqk, np.ascontiguousarray(Wv), bqk, bv


def kernel(x, W_qkv, b_qkv, W_out, b_out, _trace=False, _tmpdir=None):
    bf = ml_dtypes.bfloat16
    x = np.asarray(x, dtype=np.float32).astype(bf)
    wqk, wv, bqk, bv = _prep_weights(W_qkv, b_qkv)
    wqk = wqk.astype(bf)
    wv = wv.astype(bf)
    wout = np.ascontiguousarray(
        np.asarray(W_out, dtype=np.float32).astype(bf))
    bout = np.ascontiguousarray(np.asarray(b_out, dtype=np.float32))
    nc = _get_nc()
    in_maps = [
        {"xt": np.ascontiguousarray(x[i].T), "wqk": wqk, "wv": wv,
         "bqk": bqk, "bv": bv, "wout": wout, "bout": bout}
        for i in range(x.shape[0])
    ]
    res = run_bass_kernel_spmd(
        nc, in_maps, core_ids=list(range(x.shape[0])),
        trace=_trace, tmpdir=_tmpdir)
    outp = np.stack([rr["out"] for rr in res.results], axis=0)
    kernel.last_result = res
    return outp


# revision 33
# speedup vs baseline: 1.0668x; 1.0195x over previous
"""Multi-head attention block on 8 Trainium2 NeuronCores, data-parallel over batch.

Per core (one batch element, S=1024 seq, E=1024 embed, H=16 heads, D=64),
all matmuls in bf16 (inputs cast host-side), fp32 PSUM accumulation:
  xT fed pre-transposed from the host (feature-major [E, S])
  qT/kT = W_pair.T @ xT  (feature-major) per head-pair, pipelined as PE filler
  V     = xT.T @ Wv      (seq-major) with a ones column appended -> V_aug
  scoresT[s2,s1] = kT.T @ qT  (two heads as K=64 row-tiles, overlapped on PE)
  expT = exp(0.125*scoresT)   (ACT eviction PSUM->SBUF, softmax w/o max-subtract;
                               logits are ~N(0,1.5) so exp cannot overflow fp32)
  PV: psum[66,512] = V_aug.T @ expT  -> rows 0..63 = outT unnorm, row 64 = rowsum
  normalize: outT = psum[0:64] * broadcast(reciprocal(psum[64]))
             (fast-approx reciprocal on DVE + GPSIMD partition broadcast)
  out = outT.T @ W_out + b_out

The scalar engine (ACT) is the attention-phase co-bottleneck (16 exp
activations per pair at ~1.15us each); it is kept exp-only — all copies and
broadcasts run on DVE/GPSIMD.  Weights are de-interleaved host-side:
reference W_qkv columns are (h, d, qkv) with qkv innermost; we feed wqk
(pair-blocked [q0q1k0k1...]) and wv ((h,d) order).
"""

import ml_dtypes
import numpy as np

import concourse.bacc as bacc
import concourse.bass as bass
import concourse.mybir as mybir
from concourse.bass_utils import run_bass_kernel_spmd
from concourse.tile import TileContext
from concourse.tile_rust import add_dep_helper

F32 = mybir.dt.float32
BF16 = mybir.dt.bfloat16
AF = mybir.ActivationFunctionType

S = 1024       # sequence length
E = 1024       # embed dim
H = 16         # heads
D = 64         # head dim
P = 128        # partitions
NP = 8         # head pairs
KT = E // P    # contraction tiles (8)
SM = S // P    # seq tiles of 128 (8)
NB = S // 512  # seq banks of 512 (2)
SCALE = 1.0 / np.sqrt(D)


def build_nc():
    nc = bacc.Bacc(trn_type="TRN2", target_bir_lowering=False)
    xt = nc.dram_tensor("xt", [E, S], BF16, kind="ExternalInput")
    wqk = nc.dram_tensor("wqk", [E, 2 * E], BF16, kind="ExternalInput")
    wv = nc.dram_tensor("wv", [E, E], BF16, kind="ExternalInput")
    bqk = nc.dram_tensor("bqk", [2 * E], F32, kind="ExternalInput")
    bv = nc.dram_tensor("bv", [E], F32, kind="ExternalInput")
    wout = nc.dram_tensor("wout", [E, E], BF16, kind="ExternalInput")
    bout = nc.dram_tensor("bout", [E], F32, kind="ExternalInput")
    out = nc.dram_tensor("out", [S, E], F32, kind="ExternalOutput")

    with TileContext(nc) as tc:
        with (
            tc.tile_pool(name="const", bufs=1) as constp,
            tc.tile_pool(name="persist", bufs=1) as pers,
            tc.tile_pool(name="psum", bufs=1, space="PSUM") as psp,
        ):
            # ---- constants ----
            ones = constp.tile([1, 512], F32, tag="ones")
            nc.vector.memset(ones[:], 1.0)
            onespp = constp.tile([P, 2 * H], F32, tag="onespp")
            nc.vector.memset(onespp[:], 1.0)
            warm = constp.tile([P, 512], BF16, tag="warm")
            nc.vector.memset(warm[:], 0.0)
            # per-partition bias columns for q/k (column p*2+j is the bias for
            # pair p's q (j=0) / k (j=1) feature block)
            bcols = constp.tile([P, 2 * NP], F32, tag="bcols")

            # ---- persistent arrays ----
            xTall = pers.tile([P, KT, S], BF16, tag="xtall", name="xTall")
            xT = [xTall[:, k] for k in range(KT)]
            vaug = [pers.tile([P, H, D + 2], BF16, tag=f"va{m}", name=f"vaug{m}")
                    for m in range(SM)]
            outT = [pers.tile([P, S], BF16, tag=f"ot{p}", name=f"outT{p}")
                    for p in range(NP)]

            # broadcast biases for free-dim adds (V and final projections)
            bvb = constp.tile([P, E], F32, tag="bvb")
            boutb = constp.tile([P, E], F32, tag="boutb")
            with (
                tc.tile_pool(name="ph0", bufs=1) as ph0,
                tc.tile_pool(name="ph2", bufs=1) as ph2,
                tc.tile_pool(name="ph3", bufs=1) as ph3,
            ):
                # ---- input DMAs, spread across sync/scalar queues ----
                # Per-DMA issue costs ~0.7us on a queue, and V-proj m-tile m
                # needs xT chunk m//2 of every k plus wv bank 0 — interleave
                # so V can start ~14us and never starves afterwards.
                wvk = [[ph0.tile([P, 512], BF16, tag=f"wv{n}_{k}", name="wvk")
                        for k in range(KT)] for n in range(2)]

                def dma_xt_half(h, parity, eng):
                    ch = slice(h * 512, (h + 1) * 512)
                    for k in range(parity, KT, 2):
                        eng.dma_start(xT[k][:, ch], xt.ap()[bass.ts(k, P), ch])

                def dma_wv(n, ks, eng):
                    for k in ks:
                        eng.dma_start(
                            wvk[n][k][:], wv.ap()[bass.ts(k, P), bass.ts(n, 512)])

                dma_xt_half(0, 0, nc.sync)
                dma_xt_half(0, 1, nc.scalar)
                dma_wv(0, range(0, 4), nc.sync)
                dma_wv(0, range(4, 8), nc.scalar)
                dma_xt_half(1, 0, nc.sync)
                dma_xt_half(1, 1, nc.scalar)
                dma_wv(1, range(0, 4), nc.sync)
                dma_wv(1, range(4, 8), nc.scalar)
                # small bias DMAs on the gpsimd (SWDGE) queue
                bvr = ph0.tile([1, E], F32, tag="bvr")
                nc.gpsimd.dma_start(bvr[:], bv.ap()[None, :])
                botr = ph0.tile([1, E], F32, tag="botr")
                nc.gpsimd.dma_start(botr[:], bout.ap()[None, :])
                nc.gpsimd.dma_start(
                    bcols[:], bqk.ap().rearrange("(f p) -> p f", p=P))

                def load_wq(p):
                    wq = []
                    for k in range(KT):
                        w = ph2.tile([P, 256], BF16, tag="wqk", bufs=16, name="wqk")
                        nc.sync.dma_start(
                            w[:], wqk.ap()[bass.ts(k, P), bass.ts(p, 256)])
                        wq.append(w)
                    return wq

                wq0 = load_wq(0)

                # ---- PE warmup: junk matmuls so HAM un-throttles before V
                # and the PE never idles a full MID window while DMAs land ----
                for g in range(3):
                    wp = psp.tile([P, 512], F32, tag="mm", bufs=2, name="warmps")
                    for i in range(8):
                        nc.tensor.matmul(wp[:], warm[:, 0:P], warm[:],
                                         start=(i == 0), stop=(i == 7))

                # bias row -> all-partition broadcasts (GPSIMD, off the PE)
                nc.gpsimd.partition_broadcast(bvb[:], bvr[:])
                nc.gpsimd.partition_broadcast(boutb[:], botr[:])

                # ---- phase 1: V = x @ Wv (+bv), into vaug with ones column ----
                for m in range(SM):
                    nc.vector.tensor_copy(
                        vaug[m][:, :, D:D + 2],
                        onespp[:].rearrange("p (h t) -> p h t", h=H))
                for n in range(2):
                    for m in range(SM):
                        pv = psp.tile([P, 512], F32, tag="mm", bufs=2, name="pvps")
                        for k in range(KT):
                            nc.tensor.matmul(
                                pv[:], xT[k][:, bass.ts(m, P)], wvk[n][k][:],
                                start=(k == 0), stop=(k == KT - 1))
                        nc.vector.tensor_add(
                            vaug[m][:, bass.ts(n, 8), 0:D],
                            pv[:].rearrange("p (h d) -> p h d", h=8),
                            bvb[:, bass.ts(n, 512)].rearrange("p (h d) -> p h d", h=8))

                # ---- phase 2: attention, software-pipelined over head pairs ----
                # Iteration p computes attention for pair p while projecting
                # qt/kt for pair p+1 (proj matmuls interleaved into the scores
                # loop so PE has independent work while ACT evicts exp tiles).
                def load_wot(n):
                    cs = bass.ts(n, 512)
                    wot = []
                    for k in range(KT):
                        w = ph3.tile([P, 512], BF16, tag=f"wo{k}", bufs=2,
                                     name="wot")
                        nc.scalar.dma_start(w[:], wout.ap()[bass.ts(k, P), cs])
                        wot.append(w)
                    return wot

                def alloc_qkt():
                    qt = ph2.tile([P, S], BF16, tag="qt", bufs=2, name="qt")
                    kt = ph2.tile([P, S], BF16, tag="kt", bufs=2, name="kt")
                    return qt, kt

                def proj_mms(p, wq, qt, kt):
                    """Generator yielding groups of proj matmuls + evictions."""
                    for which in range(2):  # 0 = q, 1 = k
                        ws = slice(which * P, (which + 1) * P)
                        dst = qt if which == 0 else kt
                        bc = bcols[:, 2 * p + which:2 * p + which + 1]
                        for n in range(NB):
                            cs = bass.ts(n, 512)
                            ps = psp.tile([P, 512], F32, tag="mm", bufs=2,
                                          name="pproj")
                            for k in range(KT):
                                nc.tensor.matmul(
                                    ps[:], wq[k][:, ws], xT[k][:, cs],
                                    start=(k == 0), stop=(k == KT - 1))
                                yield
                            nc.vector.tensor_scalar_add(dst[:, cs], ps[:], bc)
                    while True:
                        yield

                def drain(gen, n):
                    for _ in range(n):
                        next(gen)

                # Final projection tiles as open/close chains: k=0..6 needs
                # only pairs 0-6 outT (ready before pair 7), so those matmuls
                # fill pair 7's attention windows; the k=7 close + bias-add +
                # store happen once pair 7's outT bank is normalized.
                _open_pf = {}

                def final_tile_open(m, n):
                    pf = psp.tile([P, 512], F32, tag="mm", bufs=2, name="pf")
                    _open_pf[(m, n)] = pf
                    wot = wot0 if n == 0 else wot1
                    for k in range(KT - 1):
                        nc.tensor.matmul(
                            pf[:], outT[k][:, bass.ts(m, P)], wot[k][:],
                            start=(k == 0), stop=False)
                        yield

                def final_tile_close(m, n):
                    pf = _open_pf.pop((m, n))
                    wot = wot0 if n == 0 else wot1
                    cs = bass.ts(n, 512)
                    nc.tensor.matmul(
                        pf[:], outT[KT - 1][:, bass.ts(m, P)], wot[KT - 1][:],
                        start=False, stop=True)
                    yield
                    osb = ph3.tile([P, 512], F32, tag="osb", bufs=3,
                                   name="osb")
                    nc.vector.tensor_add(osb[:], pf[:], boutb[:, cs])
                    nc.sync.dma_start(out.ap()[bass.ts(m, P), cs], osb[:])

                def final_b0_prefix():
                    # pair-7 bank-0 filler: two open k0-6 chains (14 matmuls)
                    yield from final_tile_open(0, 0)
                    yield from final_tile_open(1, 0)
                    while True:
                        yield

                def final_rest():
                    # after pair-7 bank-0 normalize: close the open chains,
                    # then stream the rest of the first-half tiles
                    yield from final_tile_close(0, 0)
                    yield from final_tile_close(1, 0)
                    for m, n in [(2, 0), (3, 0), (0, 1), (1, 1),
                                 (2, 1), (3, 1)]:
                        yield from final_tile_open(m, n)
                        yield from final_tile_close(m, n)
                    while True:
                        yield

                wq = wq0
                qt, kt = alloc_qkt()
                drain(proj_mms(0, wq, qt, kt), 40)

                for p in range(NP):
                    if p + 1 < NP:
                        wq_n = load_wq(p + 1)
                        qt_n, kt_n = alloc_qkt()
                        filler = proj_mms(p + 1, wq_n, qt_n, kt_n)
                    else:
                        filler = final_b0_prefix()

                    # Software pipeline over 8 groups (2 banks x 4 m-iters):
                    # group g emits scores+exp for (bank g//4, m=2(g%4)); the
                    # PV matmuls for group g-1 follow right after, so the PE
                    # queue head is always ready when PE reaches it and ACT's
                    # exp chain never starves between banks.
                    expAn = [None] * NB
                    expBn = [None] * NB
                    poAn = [None] * NB
                    poBn = [None] * NB

                    def emit_pv(n, m):
                        for j in range(2):
                            nc.tensor.matmul(
                                poAn[n][:], vaug[m + j][:, 2 * p, :],
                                expAn[n][:, m + j],
                                start=(m + j == 0), stop=(m + j == SM - 1))
                            nc.tensor.matmul(
                                poBn[n][:], vaug[m + j][:, 2 * p + 1, :],
                                expBn[n][:, m + j],
                                start=(m + j == 0), stop=(m + j == SM - 1))

                    def emit_norm(n):
                        cs = bass.ts(n, 512)
                        for h, po in ((0, poAn[n]), (1, poBn[n])):
                            # evict [66,512] to SBUF fast so the PSUM slot
                            # frees; normalize out of SBUF (DVE, not ACT —
                            # the ACT queue must stay exp-only)
                            pvt = ph2.tile([D, 512], F32, tag="pvt",
                                           bufs=4, name="pvt")
                            nc.vector.tensor_copy(pvt[:], po[0:D, :])
                            rs = ph2.tile([1, 512], F32, tag="rs", bufs=4,
                                          name="rs")
                            nc.vector.tensor_copy(rs[:], po[D:D + 1, :])
                            drain(filler, 4)
                            rec = ph2.tile([1, 512], F32, tag="rec", bufs=4,
                                           name="rec")
                            nc.vector.reciprocal_approx_fast(rec[:], rs[:])
                            rb = ph2.tile([D, 512], F32, tag="rb", bufs=4,
                                          name="rb")
                            nc.gpsimd.partition_broadcast(rb[:], rec[:])
                            nc.vector.tensor_mul(
                                outT[p][h * D:(h + 1) * D, cs],
                                pvt[:], rb[:])

                    for g in range(9):
                        if g < 8:
                            n, it = g // 4, g % 4
                            if it == 0:
                                cs = bass.ts(n, 512)
                                expAn[n] = ph2.tile([P, SM, 512], BF16,
                                                    tag="expA", bufs=2,
                                                    name="expA")
                                expBn[n] = ph2.tile([P, SM, 512], BF16,
                                                    tag="expB", bufs=2,
                                                    name="expB")
                                poAn[n] = psp.tile([D + 2, 512], F32,
                                                   tag="pv", bufs=2, name="poA")
                                poBn[n] = psp.tile([D + 2, 512], F32,
                                                   tag="pv", bufs=2, name="poB")
                            m = 2 * it
                            psA = psp.tile([P, 2, 512], F32, tag="sc", bufs=2,
                                           name="psA")
                            psB = psp.tile([P, 2, 512], F32, tag="sc", bufs=2,
                                           name="psB")
                            prev = None
                            for j in range(2):
                                ms = bass.ts(m + j, P)
                                ia = nc.tensor.matmul(
                                    psA[:, j], kt[0:D, ms], qt[0:D, cs])
                                ib = nc.tensor.matmul(
                                    psB[:, j], kt[D:P, ms], qt[D:P, cs])
                                # chain so the two half-array (row-tiled)
                                # matmuls issue back-to-back and overlap
                                if prev is not None:
                                    add_dep_helper(ia.ins, prev.ins, sync=False,
                                                   reason="pair scores order")
                                add_dep_helper(ib.ins, ia.ins, sync=False,
                                               reason="pair scores order")
                                prev = ib
                            nc.scalar.activation(
                                expAn[n][:, m:m + 2], psA[:], AF.Exp,
                                scale=SCALE)
                            nc.scalar.activation(
                                expBn[n][:, m:m + 2], psB[:], AF.Exp,
                                scale=SCALE)
                            if p == NP - 1 and g >= 4:
                                # pair 7 bank 1: drain the first-half final
                                # projection hard so none of it outlives the
                                # last exp chain
                                drain(filler, 6)
                            else:
                                drain(filler, 4 if g < 6 else 2)
                        if g >= 1:
                            pn, pit = (g - 1) // 4, (g - 1) % 4
                            emit_pv(pn, 2 * pit)
                            if pit == 3:
                                emit_norm(pn)
                                if p == NP - 1 and pn == 0:
                                    # outT bank 0 is complete for all pairs —
                                    # the first-half projection tiles become
                                    # the PE filler for pair 7's bank 1
                                    filler = final_rest()
                    if p + 1 < NP:
                        drain(filler, 64)
                        wq, qt, kt = wq_n, qt_n, kt_n
                    if p == NP - 2:
                        wot0 = load_wot(0)
                        wot1 = load_wot(1)

                # ---- phase 3 (second half; first half emitted during pair 7)
                # Keep two chains open ahead of each close: the k0-6 opens
                # depend only on pairs 0-6, so they run while pair 7's bank-1
                # normalize chain (DVE/GPSIMD) produces outT[7].
                drain(filler, 200)
                tiles = [(m, n) for m in range(SM // 2, SM) for n in range(2)]
                for _ in final_tile_open(*tiles[0]):
                    pass
                for _ in final_tile_open(*tiles[1]):
                    pass
                for i, (m, n) in enumerate(tiles):
                    for _ in final_tile_close(m, n):
                        pass
                    if i + 2 < len(tiles):
                        for _ in final_tile_open(*tiles[i + 2]):
                            pass

    nc.finalize()
    return nc


_NC = None


def _get_nc():
    global _NC
    if _NC is None:
        _NC = build_nc()
    return _NC


def _prep_weights(W_qkv, b_qkv):
    # reference column order is (h, d, qkv) with qkv innermost
    W = np.asarray(W_qkv, dtype=np.float32).reshape(E, H, D, 3)
    b = np.asarray(b_qkv, dtype=np.float32).reshape(H, D, 3)
    Wq = W[..., 0].reshape(E, E)
    Wk = W[..., 1].reshape(E, E)
    Wv = W[..., 2].reshape(E, E)
    bq = b[..., 0].reshape(E)
    bk = b[..., 1].reshape(E)
    bv = b[..., 2].reshape(E)
    wqk = np.empty((E, 2 * E), dtype=np.float32)
    bqk = np.empty(2 * E, dtype=np.float32)
    for p in range(NP):
        wqk[:, p * 256:p * 256 + P] = Wq[:, p * P:(p + 1) * P]
        wqk[:, p * 256 + P:(p + 1) * 256] = Wk[:, p * P:(p + 1) * P]
        bqk[p * 256:p * 256 + P] = bq[p * P:(p + 1) * P]
        bqk[p * 256 + P:(p + 1) * 256] = bk[p * P:(p + 1) * P]
    return wqk, np.ascontiguousarray(Wv), bqk, bv


def kernel(x, W_qkv, b_qkv, W_out, b_out, _trace=False, _tmpdir=None):
    bf = ml_dtypes.bfloat16
    x = np.asarray(x, dtype=np.float32).astype(bf)
    wqk, wv, bqk, bv = _prep_weights(W_qkv, b_qkv)
    wqk = wqk.astype(bf)
    wv = wv.astype(bf)
    wout = np.ascontiguousarray(
        np.asarray(W_out, dtype=np.float32).astype(bf))
    bout = np.ascontiguousarray(np.asarray(b_out, dtype=np.float32))
    nc = _get_nc()
    in_maps = [
        {"xt": np.ascontiguousarray(x[i].T), "wqk": wqk, "wv": wv,
         "bqk": bqk, "bv": bv, "wout": wout, "bout": bout}
        for i in range(x.shape[0])
    ]
    res = run_bass_kernel_spmd(
        nc, in_maps, core_ids=list(range(x.shape[0])),
        trace=_trace, tmpdir=_tmpdir)
    outp = np.stack([rr["out"] for rr in res.results], axis=0)
    kernel.last_result = res
    return outp


# revision 34
# speedup vs baseline: 1.0772x; 1.0098x over previous
"""Multi-head attention block on 8 Trainium2 NeuronCores, data-parallel over batch.

Per core (one batch element, S=1024 seq, E=1024 embed, H=16 heads, D=64),
all matmuls in bf16 (inputs cast host-side), fp32 PSUM accumulation:
  xT fed pre-transposed from the host (feature-major [E, S])
  qT/kT = W_pair.T @ xT  (feature-major) per head-pair, pipelined as PE filler
  V     = xT.T @ Wv      (seq-major) with a ones column appended -> V_aug
  scoresT[s2,s1] = kT.T @ qT  (two heads as K=64 row-tiles, overlapped on PE)
  expT = exp(0.125*scoresT)   (ACT eviction PSUM->SBUF, softmax w/o max-subtract;
                               logits are ~N(0,1.5) so exp cannot overflow fp32)
  PV: psum[66,512] = V_aug.T @ expT  -> rows 0..63 = outT unnorm, row 64 = rowsum
  normalize: outT = psum[0:64] * broadcast(reciprocal(psum[64]))
             (fast-approx reciprocal on DVE + GPSIMD partition broadcast)
  out = outT.T @ W_out + b_out

The scalar engine (ACT) is the attention-phase co-bottleneck (16 exp
activations per pair at ~1.15us each); it is kept exp-only — all copies and
broadcasts run on DVE/GPSIMD.  Weights are de-interleaved host-side:
reference W_qkv columns are (h, d, qkv) with qkv innermost; we feed wqk
(pair-blocked [q0q1k0k1...]) and wv ((h,d) order).
"""

import ml_dtypes
import numpy as np

import concourse.bacc as bacc
import concourse.bass as bass
import concourse.mybir as mybir
from concourse.bass_utils import run_bass_kernel_spmd
from concourse.tile import TileContext
from concourse.tile_rust import add_dep_helper

F32 = mybir.dt.float32
BF16 = mybir.dt.bfloat16
AF = mybir.ActivationFunctionType

S = 1024       # sequence length
E = 1024       # embed dim
H = 16         # heads
D = 64         # head dim
P = 128        # partitions
NP = 8         # head pairs
KT = E // P    # contraction tiles (8)
SM = S // P    # seq tiles of 128 (8)
NB = S // 512  # seq banks of 512 (2)
SCALE = 1.0 / np.sqrt(D)


def build_nc():
    nc = bacc.Bacc(trn_type="TRN2", target_bir_lowering=False)
    xt = nc.dram_tensor("xt", [E, S], BF16, kind="ExternalInput")
    wqk = nc.dram_tensor("wqk", [E, 2 * E], BF16, kind="ExternalInput")
    wv = nc.dram_tensor("wv", [E, E], BF16, kind="ExternalInput")
    bqk = nc.dram_tensor("bqk", [2 * E], F32, kind="ExternalInput")
    bv = nc.dram_tensor("bv", [E], F32, kind="ExternalInput")
    wout = nc.dram_tensor("wout", [E, E], BF16, kind="ExternalInput")
    bout = nc.dram_tensor("bout", [E], F32, kind="ExternalInput")
    out = nc.dram_tensor("out", [S, E], F32, kind="ExternalOutput")

    with TileContext(nc) as tc:
        with (
            tc.tile_pool(name="const", bufs=1) as constp,
            tc.tile_pool(name="persist", bufs=1) as pers,
            tc.tile_pool(name="psum", bufs=1, space="PSUM") as psp,
        ):
            # ---- constants ----
            ones = constp.tile([1, 512], F32, tag="ones")
            nc.vector.memset(ones[:], 1.0)
            onespp = constp.tile([P, 2 * H], F32, tag="onespp")
            nc.vector.memset(onespp[:], 1.0)
            warm = constp.tile([P, 512], BF16, tag="warm")
            nc.vector.memset(warm[:], 0.0)
            # per-partition bias columns for q/k (column p*2+j is the bias for
            # pair p's q (j=0) / k (j=1) feature block)
            bcols = constp.tile([P, 2 * NP], F32, tag="bcols")

            # ---- persistent arrays ----
            xTall = pers.tile([P, KT, S], BF16, tag="xtall", name="xTall")
            xT = [xTall[:, k] for k in range(KT)]
            vaug = [pers.tile([P, H, D + 2], BF16, tag=f"va{m}", name=f"vaug{m}")
                    for m in range(SM)]
            outT = [pers.tile([P, S], BF16, tag=f"ot{p}", name=f"outT{p}")
                    for p in range(NP)]

            # broadcast biases for free-dim adds (V and final projections)
            bvb = constp.tile([P, E], F32, tag="bvb")
            boutb = constp.tile([P, E], F32, tag="boutb")
            with (
                tc.tile_pool(name="ph0", bufs=1) as ph0,
                tc.tile_pool(name="ph2", bufs=1) as ph2,
                tc.tile_pool(name="ph3", bufs=1) as ph3,
            ):
                # ---- input DMAs, spread across sync/scalar queues ----
                # Per-DMA issue costs ~0.7us on a queue, and V-proj m-tile m
                # needs xT chunk m//2 of every k plus wv bank 0 — interleave
                # so V can start ~14us and never starves afterwards.
                wvk = [[ph0.tile([P, 512], BF16, tag=f"wv{n}_{k}", name="wvk")
                        for k in range(KT)] for n in range(2)]

                def dma_xt_half(h, parity, eng):
                    ch = slice(h * 512, (h + 1) * 512)
                    for k in range(parity, KT, 2):
                        eng.dma_start(xT[k][:, ch], xt.ap()[bass.ts(k, P), ch])

                def dma_wv(n, ks, eng):
                    for k in ks:
                        eng.dma_start(
                            wvk[n][k][:], wv.ap()[bass.ts(k, P), bass.ts(n, 512)])

                dma_xt_half(0, 0, nc.sync)
                dma_xt_half(0, 1, nc.scalar)
                dma_wv(0, range(0, 4), nc.sync)
                dma_wv(0, range(4, 8), nc.scalar)
                dma_xt_half(1, 0, nc.sync)
                dma_xt_half(1, 1, nc.scalar)
                dma_wv(1, range(0, 4), nc.sync)
                dma_wv(1, range(4, 8), nc.scalar)
                # small bias DMAs on the gpsimd (SWDGE) queue
                bvr = ph0.tile([1, E], F32, tag="bvr")
                nc.gpsimd.dma_start(bvr[:], bv.ap()[None, :])
                botr = ph0.tile([1, E], F32, tag="botr")
                nc.gpsimd.dma_start(botr[:], bout.ap()[None, :])
                nc.gpsimd.dma_start(
                    bcols[:], bqk.ap().rearrange("(f p) -> p f", p=P))

                def load_wq(p):
                    wq = []
                    for k in range(KT):
                        w = ph2.tile([P, 256], BF16, tag="wqk", bufs=16, name="wqk")
                        nc.sync.dma_start(
                            w[:], wqk.ap()[bass.ts(k, P), bass.ts(p, 256)])
                        wq.append(w)
                    return wq

                wq0 = load_wq(0)

                # ---- PE warmup: junk matmuls so HAM un-throttles before V
                # and the PE never idles a full MID window while DMAs land ----
                for g in range(3):
                    wp = psp.tile([P, 512], F32, tag="mm", bufs=2, name="warmps")
                    for i in range(8):
                        nc.tensor.matmul(wp[:], warm[:, 0:P], warm[:],
                                         start=(i == 0), stop=(i == 7))

                # bias row -> all-partition broadcasts (GPSIMD, off the PE)
                nc.gpsimd.partition_broadcast(bvb[:], bvr[:])
                nc.gpsimd.partition_broadcast(boutb[:], botr[:])

                # ---- phase 1: V = x @ Wv (+bv), into vaug with ones column ----
                for m in range(SM):
                    nc.vector.tensor_copy(
                        vaug[m][:, :, D:D + 2],
                        onespp[:].rearrange("p (h t) -> p h t", h=H))
                for n in range(2):
                    for m in range(SM):
                        pv = psp.tile([P, 512], F32, tag="mm", bufs=2, name="pvps")
                        for k in range(KT):
                            nc.tensor.matmul(
                                pv[:], xT[k][:, bass.ts(m, P)], wvk[n][k][:],
                                start=(k == 0), stop=(k == KT - 1))
                        nc.vector.tensor_add(
                            vaug[m][:, bass.ts(n, 8), 0:D],
                            pv[:].rearrange("p (h d) -> p h d", h=8),
                            bvb[:, bass.ts(n, 512)].rearrange("p (h d) -> p h d", h=8))

                # ---- phase 2: attention, software-pipelined over head pairs ----
                # Iteration p computes attention for pair p while projecting
                # qt/kt for pair p+1 (proj matmuls interleaved into the scores
                # loop so PE has independent work while ACT evicts exp tiles).
                def load_wot(n):
                    cs = bass.ts(n, 512)
                    wot = []
                    for k in range(KT):
                        w = ph3.tile([P, 512], BF16, tag=f"wo{k}", bufs=2,
                                     name="wot")
                        nc.scalar.dma_start(w[:], wout.ap()[bass.ts(k, P), cs])
                        wot.append(w)
                    return wot

                def alloc_qkt():
                    qt = ph2.tile([P, S], BF16, tag="qt", bufs=2, name="qt")
                    kt = ph2.tile([P, S], BF16, tag="kt", bufs=2, name="kt")
                    return qt, kt

                def proj_mms(p, wq, qt, kt):
                    """Generator yielding groups of proj matmuls + evictions."""
                    for which in range(2):  # 0 = q, 1 = k
                        ws = slice(which * P, (which + 1) * P)
                        dst = qt if which == 0 else kt
                        bc = bcols[:, 2 * p + which:2 * p + which + 1]
                        for n in range(NB):
                            cs = bass.ts(n, 512)
                            ps = psp.tile([P, 512], F32, tag="mm", bufs=2,
                                          name="pproj")
                            for k in range(KT):
                                nc.tensor.matmul(
                                    ps[:], wq[k][:, ws], xT[k][:, cs],
                                    start=(k == 0), stop=(k == KT - 1))
                                yield
                            nc.vector.tensor_scalar_add(dst[:, cs], ps[:], bc)
                    while True:
                        yield

                def drain(gen, n):
                    for _ in range(n):
                        next(gen)

                # Final projection tiles as open/close chains: k=0..6 needs
                # only pairs 0-6 outT (ready before pair 7), so those matmuls
                # fill pair 7's attention windows; the k=7 close + bias-add +
                # store happen once pair 7's outT bank is normalized.
                _open_pf = {}

                def final_tile_open(m, n):
                    pf = psp.tile([P, 512], F32, tag="mm", bufs=2, name="pf")
                    _open_pf[(m, n)] = pf
                    wot = wot0 if n == 0 else wot1
                    for k in range(KT - 1):
                        nc.tensor.matmul(
                            pf[:], outT[k][:, bass.ts(m, P)], wot[k][:],
                            start=(k == 0), stop=False)
                        yield

                def final_tile_close(m, n):
                    pf = _open_pf.pop((m, n))
                    wot = wot0 if n == 0 else wot1
                    cs = bass.ts(n, 512)
                    nc.tensor.matmul(
                        pf[:], outT[KT - 1][:, bass.ts(m, P)], wot[KT - 1][:],
                        start=False, stop=True)
                    yield
                    osb = ph3.tile([P, 512], F32, tag="osb", bufs=3,
                                   name="osb")
                    nc.vector.tensor_add(osb[:], pf[:], boutb[:, cs])
                    nc.sync.dma_start(out.ap()[bass.ts(m, P), cs], osb[:])

                def final_b0_prefix():
                    # pair-7 bank-0 filler: two open k0-6 chains (14 matmuls)
                    yield from final_tile_open(0, 0)
                    yield from final_tile_open(1, 0)
                    while True:
                        yield

                def final_rest():
                    # after pair-7 bank-0 normalize: close the open chains,
                    # then stream the rest of the first-half tiles
                    yield from final_tile_close(0, 0)
                    yield from final_tile_close(1, 0)
                    for m, n in [(2, 0), (3, 0), (0, 1), (1, 1),
                                 (2, 1), (3, 1)]:
                        yield from final_tile_open(m, n)
                        yield from final_tile_close(m, n)
                    while True:
                        yield

                wq = wq0
                qt, kt = alloc_qkt()
                drain(proj_mms(0, wq, qt, kt), 40)

                # Global software pipeline over all 64 (pair, bank, m-iter)
                # groups — PV lags scores by one group and pair boundaries
                # are just another group transition, so neither the PE queue
                # nor ACT's exp chain ever resets between pairs.
                exp_t = {}
                po_t = {}
                qk = {0: (qt, kt)}

                def emit_pv(pp, pn, m):
                    eA, eB = exp_t[(pp, pn)]
                    pA, pB = po_t[(pp, pn)]
                    for j in range(2):
                        nc.tensor.matmul(
                            pA[:], vaug[m + j][:, 2 * pp, :], eA[:, m + j],
                            start=(m + j == 0), stop=(m + j == SM - 1))
                        nc.tensor.matmul(
                            pB[:], vaug[m + j][:, 2 * pp + 1, :], eB[:, m + j],
                            start=(m + j == 0), stop=(m + j == SM - 1))

                def emit_norm(pp, pn):
                    cs = bass.ts(pn, 512)
                    pA, pB = po_t.pop((pp, pn))
                    exp_t.pop((pp, pn))
                    for h, po in ((0, pA), (1, pB)):
                        # evict [66,512] to SBUF fast so the PSUM slot
                        # frees; normalize out of SBUF (DVE, not ACT —
                        # the ACT queue must stay exp-only)
                        pvt = ph2.tile([D, 512], F32, tag="pvt",
                                       bufs=4, name="pvt")
                        nc.vector.tensor_copy(pvt[:], po[0:D, :])
                        rs = ph2.tile([1, 512], F32, tag="rs", bufs=4,
                                      name="rs")
                        nc.vector.tensor_copy(rs[:], po[D:D + 1, :])
                        drain(filler, 4)
                        rec = ph2.tile([1, 512], F32, tag="rec", bufs=4,
                                       name="rec")
                        nc.vector.reciprocal_approx_fast(rec[:], rs[:])
                        rb = ph2.tile([D, 512], F32, tag="rb", bufs=4,
                                      name="rb")
                        nc.gpsimd.partition_broadcast(rb[:], rec[:])
                        nc.vector.tensor_mul(
                            outT[pp][h * D:(h + 1) * D, cs],
                            pvt[:], rb[:])

                NG = NP * 8
                for G in range(NG + 1):
                    if G < NG:
                        p, g = G // 8, G % 8
                        n, it = g // 4, g % 4
                        if g == 0:
                            if p + 1 < NP:
                                wq_n = load_wq(p + 1)
                                qtkt = alloc_qkt()
                                qk[p + 1] = qtkt
                                filler = proj_mms(p + 1, wq_n, *qtkt)
                            else:
                                filler = final_b0_prefix()
                            if p == NP - 2:
                                wot0 = load_wot(0)
                                wot1 = load_wot(1)
                            qt, kt = qk.pop(p)
                        if it == 0:
                            cs = bass.ts(n, 512)
                            exp_t[(p, n)] = (
                                ph2.tile([P, SM, 512], BF16, tag="expA",
                                         bufs=2, name="expA"),
                                ph2.tile([P, SM, 512], BF16, tag="expB",
                                         bufs=2, name="expB"))
                            po_t[(p, n)] = (
                                psp.tile([D + 2, 512], F32, tag="pv",
                                         bufs=2, name="poA"),
                                psp.tile([D + 2, 512], F32, tag="pv",
                                         bufs=2, name="poB"))
                        eA, eB = exp_t[(p, n)]
                        m = 2 * it
                        psA = psp.tile([P, 2, 512], F32, tag="sc", bufs=2,
                                       name="psA")
                        psB = psp.tile([P, 2, 512], F32, tag="sc", bufs=2,
                                       name="psB")
                        prev = None
                        for j in range(2):
                            ms = bass.ts(m + j, P)
                            ia = nc.tensor.matmul(
                                psA[:, j], kt[0:D, ms], qt[0:D, cs])
                            ib = nc.tensor.matmul(
                                psB[:, j], kt[D:P, ms], qt[D:P, cs])
                            # chain so the two half-array (row-tiled)
                            # matmuls issue back-to-back and overlap
                            if prev is not None:
                                add_dep_helper(ia.ins, prev.ins, sync=False,
                                               reason="pair scores order")
                            add_dep_helper(ib.ins, ia.ins, sync=False,
                                           reason="pair scores order")
                            prev = ib
                        nc.scalar.activation(
                            eA[:, m:m + 2], psA[:], AF.Exp, scale=SCALE)
                        nc.scalar.activation(
                            eB[:, m:m + 2], psB[:], AF.Exp, scale=SCALE)
                        if p == NP - 1 and g >= 4:
                            # pair 7 bank 1: drain the first-half final
                            # projection hard so none of it outlives the
                            # last exp chain
                            drain(filler, 6)
                        else:
                            drain(filler, 4 if g < 6 else 2)
                    if G >= 1:
                        Gp = G - 1
                        pp, gg = Gp // 8, Gp % 8
                        pn, pit = gg // 4, gg % 4
                        emit_pv(pp, pn, 2 * pit)
                        if pit == 3:
                            emit_norm(pp, pn)
                            if pp == NP - 1 and pn == 0:
                                # outT bank 0 is complete for all pairs —
                                # the first-half projection tiles become
                                # the PE filler for pair 7's bank 1
                                filler = final_rest()

                # ---- phase 3 (second half; first half emitted during pair 7)
                # Keep two chains open ahead of each close: the k0-6 opens
                # depend only on pairs 0-6, so they run while pair 7's bank-1
                # normalize chain (DVE/GPSIMD) produces outT[7].
                drain(filler, 200)
                tiles = [(m, n) for m in range(SM // 2, SM) for n in range(2)]
                for _ in final_tile_open(*tiles[0]):
                    pass
                for _ in final_tile_open(*tiles[1]):
                    pass
                for i, (m, n) in enumerate(tiles):
                    for _ in final_tile_close(m, n):
                        pass
                    if i + 2 < len(tiles):
                        for _ in final_tile_open(*tiles[i + 2]):
                            pass

    nc.finalize()
    return nc


_NC = None


def _get_nc():
    global _NC
    if _NC is None:
        _NC = build_nc()
    return _NC


def _prep_weights(W_qkv, b_qkv):
    # reference column order is (h, d, qkv) with qkv innermost
    W = np.asarray(W_qkv, dtype=np.float32).reshape(E, H, D, 3)
    b = np.asarray(b_qkv, dtype=np.float32).reshape(H, D, 3)
    Wq = W[..., 0].reshape(E, E)
    Wk = W[..., 1].reshape(E, E)
    Wv = W[..., 2].reshape(E, E)
    bq = b[..., 0].reshape(E)
    bk = b[..., 1].reshape(E)
    bv = b[..., 2].reshape(E)
    wqk = np.empty((E, 2 * E), dtype=np.float32)
    bqk = np.empty(2 * E, dtype=np.float32)
    for p in range(NP):
        wqk[:, p * 256:p * 256 + P] = Wq[:, p * P:(p + 1) * P]
        wqk[:, p * 256 + P:(p + 1) * 256] = Wk[:, p * P:(p + 1) * P]
        bqk[p * 256:p * 256 + P] = bq[p * P:(p + 1) * P]
        bqk[p * 256 + P:(p + 1) * 256] = bk[p * P:(p + 1) * P]
    return wqk, np.ascontiguousarray(Wv), bqk, bv


def kernel(x, W_qkv, b_qkv, W_out, b_out, _trace=False, _tmpdir=None):
    bf = ml_dtypes.bfloat16
    x = np.asarray(x, dtype=np.float32).astype(bf)
    wqk, wv, bqk, bv = _prep_weights(W_qkv, b_qkv)
    wqk = wqk.astype(bf)
    wv = wv.astype(bf)
    wout = np.ascontiguousarray(
        np.asarray(W_out, dtype=np.float32).astype(bf))
    bout = np.ascontiguousarray(np.asarray(b_out, dtype=np.float32))
    nc = _get_nc()
    in_maps = [
        {"xt": np.ascontiguousarray(x[i].T), "wqk": wqk, "wv": wv,
         "bqk": bqk, "bv": bv, "wout": wout, "bout": bout}
        for i in range(x.shape[0])
    ]
    res = run_bass_kernel_spmd(
        nc, in_maps, core_ids=list(range(x.shape[0])),
        trace=_trace, tmpdir=_tmpdir)
    outp = np.stack([rr["out"] for rr in res.results], axis=0)
    kernel.last_result = res
    return outp


# revision 38
# speedup vs baseline: 1.0942x; 1.0157x over previous
"""Multi-head attention block on 8 Trainium2 NeuronCores, data-parallel over batch.

Per core (one batch element, S=1024 seq, E=1024 embed, H=16 heads, D=64),
all matmuls in bf16 (inputs cast host-side), fp32 PSUM accumulation:
  xT fed pre-transposed from the host (feature-major [E, S])
  qT/kT = W_pair.T @ xT  (feature-major) per head-pair, pipelined as PE filler
  V     = xT.T @ Wv      (seq-major) with a ones column appended -> V_aug
  scoresT[s2,s1] = kT.T @ qT  (two heads as K=64 row-tiles, overlapped on PE)
  expT = exp(0.125*scoresT)   (ACT eviction PSUM->SBUF, softmax w/o max-subtract;
                               logits are ~N(0,1.5) so exp cannot overflow fp32)
  PV: psum[66,512] = V_aug.T @ expT  -> rows 0..63 = outT unnorm, row 64 = rowsum
  normalize: outT = psum[0:64] * broadcast(reciprocal(psum[64]))
             (fast-approx reciprocal on DVE + GPSIMD partition broadcast)
  out = outT.T @ W_out + b_out

The scalar engine (ACT) is the attention-phase co-bottleneck (16 exp
activations per pair at ~1.15us each); it is kept exp-only — all copies and
broadcasts run on DVE/GPSIMD.  Weights are de-interleaved host-side:
reference W_qkv columns are (h, d, qkv) with qkv innermost; we feed wqk
(pair-blocked [q0q1k0k1...]) and wv ((h,d) order).
"""

import ml_dtypes
import numpy as np

import concourse.bacc as bacc
import concourse.bass as bass
import concourse.mybir as mybir
from concourse.bass_utils import run_bass_kernel_spmd
from concourse.tile import TileContext
from concourse.tile_rust import add_dep_helper

F32 = mybir.dt.float32
BF16 = mybir.dt.bfloat16
AF = mybir.ActivationFunctionType

S = 1024       # sequence length
E = 1024       # embed dim
H = 16         # heads
D = 64         # head dim
P = 128        # partitions
NP = 8         # head pairs
KT = E // P    # contraction tiles (8)
SM = S // P    # seq tiles of 128 (8)
NB = S // 512  # seq banks of 512 (2)
SCALE = 1.0 / np.sqrt(D)


def build_nc():
    nc = bacc.Bacc(trn_type="TRN2", target_bir_lowering=False)
    xt = nc.dram_tensor("xt", [E, S], BF16, kind="ExternalInput")
    wqk = nc.dram_tensor("wqk", [E, 2 * E], BF16, kind="ExternalInput")
    wv = nc.dram_tensor("wv", [E, E], BF16, kind="ExternalInput")
    bqk = nc.dram_tensor("bqk", [2 * E], F32, kind="ExternalInput")
    bv = nc.dram_tensor("bv", [E], F32, kind="ExternalInput")
    wout = nc.dram_tensor("wout", [E, E], BF16, kind="ExternalInput")
    bout = nc.dram_tensor("bout", [E], F32, kind="ExternalInput")
    out = nc.dram_tensor("out", [S, E], F32, kind="ExternalOutput")

    with TileContext(nc) as tc:
        with (
            tc.tile_pool(name="const", bufs=1) as constp,
            tc.tile_pool(name="persist", bufs=1) as pers,
            tc.tile_pool(name="psum", bufs=1, space="PSUM") as psp,
        ):
            # ---- constants ----
            ones = constp.tile([1, 512], F32, tag="ones")
            nc.vector.memset(ones[:], 1.0)
            onespp = constp.tile([P, 2 * H], F32, tag="onespp")
            nc.vector.memset(onespp[:], 1.0)
            warm = constp.tile([P, 512], BF16, tag="warm")
            nc.vector.memset(warm[:], 0.0)
            # per-partition bias columns for q/k (column p*2+j is the bias for
            # pair p's q (j=0) / k (j=1) feature block)
            bcols = constp.tile([P, 2 * NP], F32, tag="bcols")

            # ---- persistent arrays ----
            xTall = pers.tile([P, KT, S], BF16, tag="xtall", name="xTall")
            xT = [xTall[:, k] for k in range(KT)]
            vaug = [pers.tile([P, H, D + 2], BF16, tag=f"va{m}", name=f"vaug{m}")
                    for m in range(SM)]
            outT = [pers.tile([P, S], BF16, tag=f"ot{p}", name=f"outT{p}")
                    for p in range(NP)]

            # broadcast biases for free-dim adds (V and final projections)
            bvb = constp.tile([P, E], F32, tag="bvb")
            boutb = constp.tile([P, E], F32, tag="boutb")
            with (
                tc.tile_pool(name="ph0", bufs=1) as ph0,
                tc.tile_pool(name="ph2", bufs=1) as ph2,
                tc.tile_pool(name="ph3", bufs=1) as ph3,
            ):
                # ---- input DMAs, spread across sync/scalar queues ----
                # Per-DMA issue costs ~0.7us on a queue, and V-proj m-tile m
                # needs xT chunk m//2 of every k plus wv bank 0 — interleave
                # so V can start ~14us and never starves afterwards.
                wvk = [[ph0.tile([P, 512], BF16, tag=f"wv{n}_{k}", name="wvk")
                        for k in range(KT)] for n in range(2)]

                def dma_xt_half(h, parity, eng):
                    ch = slice(h * 512, (h + 1) * 512)
                    for k in range(parity, KT, 2):
                        eng.dma_start(xT[k][:, ch], xt.ap()[bass.ts(k, P), ch])

                def dma_wv(n, ks, eng):
                    for k in ks:
                        eng.dma_start(
                            wvk[n][k][:], wv.ap()[bass.ts(k, P), bass.ts(n, 512)])

                dma_xt_half(0, 0, nc.sync)
                dma_xt_half(0, 1, nc.scalar)
                dma_wv(0, range(0, 4), nc.sync)
                dma_wv(0, range(4, 8), nc.scalar)
                dma_xt_half(1, 0, nc.sync)
                dma_xt_half(1, 1, nc.scalar)
                dma_wv(1, range(0, 4), nc.sync)
                dma_wv(1, range(4, 8), nc.scalar)
                # small bias DMAs on the gpsimd (SWDGE) queue
                bvr = ph0.tile([1, E], F32, tag="bvr")
                nc.gpsimd.dma_start(bvr[:], bv.ap()[None, :])
                botr = ph0.tile([1, E], F32, tag="botr")
                nc.gpsimd.dma_start(botr[:], bout.ap()[None, :])
                nc.gpsimd.dma_start(
                    bcols[:], bqk.ap().rearrange("(f p) -> p f", p=P))

                def load_wq(p):
                    wq = []
                    for k in range(KT):
                        w = ph2.tile([P, 256], BF16, tag="wqk", bufs=16, name="wqk")
                        nc.sync.dma_start(
                            w[:], wqk.ap()[bass.ts(k, P), bass.ts(p, 256)])
                        wq.append(w)
                    return wq

                wq0 = load_wq(0)

                # ---- PE warmup: junk matmuls so HAM un-throttles before V
                # and the PE never idles a full MID window while DMAs land ----
                for g in range(3):
                    wp = psp.tile([P, 512], F32, tag="mm", bufs=2, name="warmps")
                    for i in range(8):
                        nc.tensor.matmul(wp[:], warm[:, 0:P], warm[:],
                                         start=(i == 0), stop=(i == 7))

                # bias row -> all-partition broadcasts (GPSIMD, off the PE)
                nc.gpsimd.partition_broadcast(bvb[:], bvr[:])
                nc.gpsimd.partition_broadcast(boutb[:], botr[:])

                # ---- phase 1: V = x @ Wv (+bv), into vaug with ones column ----
                for m in range(SM):
                    nc.vector.tensor_copy(
                        vaug[m][:, :, D:D + 2],
                        onespp[:].rearrange("p (h t) -> p h t", h=H))
                for n in range(2):
                    for m in range(SM):
                        pv = psp.tile([P, 512], F32, tag="mm", bufs=2, name="pvps")
                        for k in range(KT):
                            nc.tensor.matmul(
                                pv[:], xT[k][:, bass.ts(m, P)], wvk[n][k][:],
                                start=(k == 0), stop=(k == KT - 1))
                        nc.vector.tensor_add(
                            vaug[m][:, bass.ts(n, 8), 0:D],
                            pv[:].rearrange("p (h d) -> p h d", h=8),
                            bvb[:, bass.ts(n, 512)].rearrange("p (h d) -> p h d", h=8))

                # ---- phase 2: attention, software-pipelined over head pairs ----
                # Iteration p computes attention for pair p while projecting
                # qt/kt for pair p+1 (proj matmuls interleaved into the scores
                # loop so PE has independent work while ACT evicts exp tiles).
                def load_wot(n):
                    cs = bass.ts(n, 512)
                    wot = []
                    for k in range(KT):
                        w = ph3.tile([P, 512], BF16, tag=f"wo{k}", bufs=2,
                                     name="wot")
                        nc.scalar.dma_start(w[:], wout.ap()[bass.ts(k, P), cs])
                        wot.append(w)
                    return wot

                def alloc_qkt():
                    qt = ph2.tile([P, S], BF16, tag="qt", bufs=2, name="qt")
                    kt = ph2.tile([P, S], BF16, tag="kt", bufs=2, name="kt")
                    return qt, kt

                def proj_mms(p, wq, qt, kt):
                    """Generator yielding groups of proj matmuls + evictions."""
                    for which in range(2):  # 0 = q, 1 = k
                        ws = slice(which * P, (which + 1) * P)
                        dst = qt if which == 0 else kt
                        bc = bcols[:, 2 * p + which:2 * p + which + 1]
                        for n in range(NB):
                            cs = bass.ts(n, 512)
                            ps = psp.tile([P, 512], F32, tag="mm", bufs=2,
                                          name="pproj")
                            for k in range(KT):
                                nc.tensor.matmul(
                                    ps[:], wq[k][:, ws], xT[k][:, cs],
                                    start=(k == 0), stop=(k == KT - 1))
                                yield
                            nc.vector.tensor_scalar_add(dst[:, cs], ps[:], bc)
                    while True:
                        yield

                def drain(gen, n):
                    for _ in range(n):
                        next(gen)

                # Final projection tiles as open/close chains: k=0..6 needs
                # only pairs 0-6 outT (ready before pair 7), so those matmuls
                # fill pair 7's attention windows; the k=7 close + bias-add +
                # store happen once pair 7's outT bank is normalized.
                _open_pf = {}

                def final_tile_open(m, n, tag="mm"):
                    pf = psp.tile([P, 512], F32, tag=tag, bufs=2, name="pf")
                    _open_pf[(m, n)] = pf
                    wot = wot0 if n == 0 else wot1
                    for k in range(KT - 1):
                        nc.tensor.matmul(
                            pf[:], outT[k][:, bass.ts(m, P)], wot[k][:],
                            start=(k == 0), stop=False)
                        yield

                def final_tile_close(m, n):
                    pf = _open_pf.pop((m, n))
                    wot = wot0 if n == 0 else wot1
                    cs = bass.ts(n, 512)
                    nc.tensor.matmul(
                        pf[:], outT[KT - 1][:, bass.ts(m, P)], wot[KT - 1][:],
                        start=False, stop=True)
                    yield
                    osb = ph3.tile([P, 512], F32, tag="osb", bufs=3,
                                   name="osb")
                    nc.vector.tensor_add(osb[:], pf[:], boutb[:, cs])
                    nc.sync.dma_start(out.ap()[bass.ts(m, P), cs], osb[:])

                def final_b0_prefix():
                    # pair-7 bank-0 filler: two open k0-6 chains (14 matmuls)
                    yield from final_tile_open(0, 0)
                    yield from final_tile_open(1, 0)
                    while True:
                        yield

                def final_rest():
                    # after pair-7 bank-0 normalize: close the open chains,
                    # then stream the rest of the first-half tiles
                    yield from final_tile_close(0, 0)
                    yield from final_tile_close(1, 0)
                    for m, n in [(2, 0), (3, 0), (0, 1), (1, 1),
                                 (2, 1), (3, 1)]:
                        yield from final_tile_open(m, n)
                        yield from final_tile_close(m, n)
                    while True:
                        yield

                wq = wq0
                qt, kt = alloc_qkt()
                drain(proj_mms(0, wq, qt, kt), 40)

                # Global software pipeline over all 64 (pair, bank, m-iter)
                # groups — PV lags scores by one group and pair boundaries
                # are just another group transition, so neither the PE queue
                # nor ACT's exp chain ever resets between pairs.
                exp_t = {}
                po_t = {}
                qk = {0: (qt, kt)}

                def emit_pv(pp, pn, m):
                    eA, eB = exp_t[(pp, pn)]
                    pA, pB = po_t[(pp, pn)]
                    for j in range(2):
                        nc.tensor.matmul(
                            pA[:], vaug[m + j][:, 2 * pp, :], eA[:, m + j],
                            start=(m + j == 0), stop=(m + j == SM - 1))
                        nc.tensor.matmul(
                            pB[:], vaug[m + j][:, 2 * pp + 1, :], eB[:, m + j],
                            start=(m + j == 0), stop=(m + j == SM - 1))

                def emit_norm(pp, pn):
                    # evict [66,512] to SBUF fast so the PSUM slots free;
                    # normalize out of SBUF (DVE, not ACT — the ACT queue
                    # must stay exp-only).  Both heads' chains are
                    # interleaved so the two GPSIMD broadcasts overlap the
                    # DVE work and outT[pp] completes sooner.
                    cs = bass.ts(pn, 512)
                    pA, pB = po_t.pop((pp, pn))
                    exp_t.pop((pp, pn))
                    pvt = [None, None]
                    rec = [None, None]
                    rb = [None, None]
                    for h, po in ((0, pA), (1, pB)):
                        pvt[h] = ph2.tile([D, 512], F32, tag="pvt",
                                          bufs=4, name="pvt")
                        nc.vector.tensor_copy(pvt[h][:], po[0:D, :])
                        rs = ph2.tile([1, 512], F32, tag="rs", bufs=4,
                                      name="rs")
                        nc.vector.tensor_copy(rs[:], po[D:D + 1, :])
                        rec[h] = ph2.tile([1, 512], F32, tag="rec", bufs=4,
                                          name="rec")
                        nc.vector.reciprocal_approx_fast(rec[h][:], rs[:])
                        drain(filler, 2)
                    for h in range(2):
                        rb[h] = ph2.tile([D, 512], F32, tag="rb", bufs=4,
                                         name="rb")
                        nc.gpsimd.partition_broadcast(rb[h][:], rec[h][:])
                        drain(filler, 2)
                    for h in range(2):
                        nc.vector.tensor_mul(
                            outT[pp][h * D:(h + 1) * D, cs],
                            pvt[h][:], rb[h][:])

                NG = NP * 8
                for G in range(NG + 1):
                    if G < NG:
                        p, g = G // 8, G % 8
                        n, it = g // 4, g % 4
                        if g == 0:
                            if p + 1 < NP:
                                wq_n = load_wq(p + 1)
                                qtkt = alloc_qkt()
                                qk[p + 1] = qtkt
                                filler = proj_mms(p + 1, wq_n, *qtkt)
                            else:
                                filler = final_b0_prefix()
                            if p == NP - 2:
                                wot0 = load_wot(0)
                                wot1 = load_wot(1)
                            qt, kt = qk.pop(p)
                        if it == 0:
                            cs = bass.ts(n, 512)
                            exp_t[(p, n)] = (
                                ph2.tile([P, SM, 512], BF16, tag="expA",
                                         bufs=2, name="expA"),
                                ph2.tile([P, SM, 512], BF16, tag="expB",
                                         bufs=2, name="expB"))
                            po_t[(p, n)] = (
                                psp.tile([D + 2, 512], F32, tag="pv",
                                         bufs=2, name="poA"),
                                psp.tile([D + 2, 512], F32, tag="pv",
                                         bufs=2, name="poB"))
                        eA, eB = exp_t[(p, n)]
                        m = 2 * it
                        psA = psp.tile([P, 2, 512], F32, tag="sc", bufs=2,
                                       name="psA")
                        psB = psp.tile([P, 2, 512], F32, tag="sc", bufs=2,
                                       name="psB")
                        prev = None
                        for j in range(2):
                            ms = bass.ts(m + j, P)
                            ia = nc.tensor.matmul(
                                psA[:, j], kt[0:D, ms], qt[0:D, cs])
                            ib = nc.tensor.matmul(
                                psB[:, j], kt[D:P, ms], qt[D:P, cs])
                            # chain so the two half-array (row-tiled)
                            # matmuls issue back-to-back and overlap
                            if prev is not None:
                                add_dep_helper(ia.ins, prev.ins, sync=False,
                                               reason="pair scores order")
                            add_dep_helper(ib.ins, ia.ins, sync=False,
                                           reason="pair scores order")
                            prev = ib
                        nc.scalar.activation(
                            eA[:, m:m + 2], psA[:], AF.Exp, scale=SCALE)
                        nc.scalar.activation(
                            eB[:, m:m + 2], psB[:], AF.Exp, scale=SCALE)
                        if p == NP - 1 and g >= 4:
                            # pair 7 bank 1: drain the first-half final
                            # projection hard so none of it outlives the
                            # last exp chain
                            drain(filler, 6 if g == 4 else 8)
                        else:
                            drain(filler, 4 if g < 6 else 2)
                    if G >= 1:
                        Gp = G - 1
                        pp, gg = Gp // 8, Gp % 8
                        pn, pit = gg // 4, gg % 4
                        emit_pv(pp, pn, 2 * pit)
                        if pit == 3:
                            emit_norm(pp, pn)
                            if pp == NP - 1 and pn == 0:
                                # outT bank 0 is complete for all pairs —
                                # the first-half projection tiles become
                                # the PE filler for pair 7's bank 1
                                filler = final_rest()

                # ---- phase 3 (second half; first half emitted during pair 7)
                # Keep two chains open ahead of each close: the k0-6 opens
                # depend only on pairs 0-6, so they run while pair 7's bank-1
                # normalize chain (DVE/GPSIMD) produces outT[7].
                drain(filler, 200)
                # The scores PSUM buffers are idle now — alternate the final
                # chains between the 'sc' and 'mm' tags so four accumulators
                # rotate and the DVE bias-add evictions never gate an open.
                tiles = [(m, n) for m in range(SM // 2, SM) for n in range(2)]
                tags = ["sc" if i % 2 == 0 else "mm" for i in range(len(tiles))]
                for i in range(2):
                    for _ in final_tile_open(*tiles[i], tag=tags[i]):
                        pass
                for i, (m, n) in enumerate(tiles):
                    for _ in final_tile_close(m, n):
                        pass
                    if i + 2 < len(tiles):
                        for _ in final_tile_open(*tiles[i + 2], tag=tags[i + 2]):
                            pass

    nc.finalize()
    return nc


_NC = None


def _get_nc():
    global _NC
    if _NC is None:
        _NC = build_nc()
    return _NC


def _prep_weights(W_qkv, b_qkv):
    # reference column order is (h, d, qkv) with qkv innermost
    W = np.asarray(W_qkv, dtype=np.float32).reshape(E, H, D, 3)
    b = np.asarray(b_qkv, dtype=np.float32).reshape(H, D, 3)
    Wq = W[..., 0].reshape(E, E)
    Wk = W[..., 1].reshape(E, E)
    Wv = W[..., 2].reshape(E, E)
    bq = b[..., 0].reshape(E)
    bk = b[..., 1].reshape(E)
    bv = b[..., 2].reshape(E)
    wqk = np.empty((E, 2 * E), dtype=np.float32)
    bqk = np.empty(2 * E, dtype=np.float32)
    for p in range(NP):
        wqk[:, p * 256:p * 256 + P] = Wq[:, p * P:(p + 1) * P]
        wqk[:, p * 256 + P:(p + 1) * 256] = Wk[:, p * P:(p + 1) * P]
        bqk[p * 256:p * 256 + P] = bq[p * P:(p + 1) * P]
        bqk[p * 256 + P:(p + 1) * 256] = bk[p * P:(p + 1) * P]
    return wqk, np.ascontiguousarray(Wv), bqk, bv


def kernel(x, W_qkv, b_qkv, W_out, b_out, _trace=False, _tmpdir=None):
    bf = ml_dtypes.bfloat16
    x = np.asarray(x, dtype=np.float32).astype(bf)
    wqk, wv, bqk, bv = _prep_weights(W_qkv, b_qkv)
    wqk = wqk.astype(bf)
    wv = wv.astype(bf)
    wout = np.ascontiguousarray(
        np.asarray(W_out, dtype=np.float32).astype(bf))
    bout = np.ascontiguousarray(np.asarray(b_out, dtype=np.float32))
    nc = _get_nc()
    in_maps = [
        {"xt": np.ascontiguousarray(x[i].T), "wqk": wqk, "wv": wv,
         "bqk": bqk, "bv": bv, "wout": wout, "bout": bout}
        for i in range(x.shape[0])
    ]
    res = run_bass_kernel_spmd(
        nc, in_maps, core_ids=list(range(x.shape[0])),
        trace=_trace, tmpdir=_tmpdir)
    outp = np.stack([rr["out"] for rr in res.results], axis=0)
    kernel.last_result = res
    return outp
